# revision 12
# baseline (speedup 1.0000x reference)
"""DeepSeekMoE Trainium2 kernel: expert-parallel across 8 NeuronCores.

Strategy:
  - Host computes routing (3 small sigmoid routers + top-k) and performs the
    token all-to-all: for each expert e (= core e), gather the tokens that
    chose e in their top-2, grouped by their top-1 sub-expert, padded to a
    static capacity.  Tokens ship transposed ([H, tokens]) so the two matmul
    chains need no on-chip transposes:
        H1^T = W1^T @ X^T   (gelu + b1 fused on ScalarE)
        Y^T  = W2^T @ H1^T
    Weights are the stationary PE operand in their natural [in, out] layout.
  - The shared-expert path is data-parallel: core c processes tokens
    [c*256, (c+1)*256), grouped by top-1 shared sub-expert.
  - Host applies the second bias + sigmoid gates and scatter-adds back.
Only the routing/dispatch (<1% of FLOPs) runs on host; both FFN chains for
every selected (expert, sub-expert) combo run on device.
"""

import sys

sys.path.insert(0, "/opt/trn_rl_repo")

import numpy as np

import concourse.bass as bass  # noqa: F401  (registers AP machinery)
import concourse.mybir as mybir
from concourse import bacc
from concourse.tile import TileContext
from concourse.bass_utils import run_bass_kernel_spmd

N, H, E, S = 2048, 512, 8, 4
F_SH = 128
P = 128
NCORES = 8
N_PER_CORE = N // NCORES

# Device compute dtype for matmul operands.  fp16 runs the PE at 1 cycle/row
# (4x faster than fp32) with fp32 PSUM accumulation; with |x|<~6 and |w|<~0.2
# there is no overflow risk and ~2^-11 mantissa keeps rel-err ~1e-4.
MM_DT = mybir.dt.float16
OUT_DT = mybir.dt.float16  # device->host y dtype (fp32 PSUM quantized once)

_compiled_cache: dict = {}
_last_res = None


def _roundup(v: int, m: int) -> int:
    return ((v + m - 1) // m) * m


def _np_dt(dt):
    return mybir.dt.np(dt)


def _build_module(capR: int, capS: int):
    """Build + compile the SPMD per-core Bass module."""
    GR = S * capR
    GS = S * capS
    kt = H // P  # 4 k-tiles over the hidden dim

    nc = bacc.Bacc("TRN2", target_bir_lowering=False, debug=False)
    dt = MM_DT

    xr = nc.dram_tensor("xr", [H, GR], dt, kind="ExternalInput").ap()
    xs = nc.dram_tensor("xs", [H, GS], dt, kind="ExternalInput").ap()
    w1 = nc.dram_tensor("w1", [S, H, H], dt, kind="ExternalInput").ap()
    w2 = nc.dram_tensor("w2", [S, H, H], dt, kind="ExternalInput").ap()
    sw1 = nc.dram_tensor("sw1", [S, H, F_SH], dt, kind="ExternalInput").ap()
    sw2 = nc.dram_tensor("sw2", [S, F_SH, H], dt, kind="ExternalInput").ap()
    # b1 packed on host: b1p[p, s*kt + f] = b1[s, f*128 + p]
    b1p = nc.dram_tensor("b1p", [P, S * kt], mybir.dt.float32, kind="ExternalInput").ap()
    # sb1 packed: sb1p[p, s] = sb1[s, p]
    sb1p = nc.dram_tensor("sb1p", [P, S], mybir.dt.float32, kind="ExternalInput").ap()

    yr = nc.dram_tensor("yr", [H, GR], OUT_DT, kind="ExternalOutput").ap()
    ys = nc.dram_tensor("ys", [H, GS], OUT_DT, kind="ExternalOutput").ap()

    GELU = mybir.ActivationFunctionType.Gelu

    with TileContext(nc) as tc:
        with (
            tc.tile_pool(name="weights", bufs=1) as wpool,
            tc.tile_pool(name="acts", bufs=1) as apool,
            tc.tile_pool(name="outs", bufs=2) as opool,
            tc.tile_pool(name="psum", bufs=4, space="PSUM") as ppool,
        ):
            # ---- loads: one dma_start per tensor/group (the ~600ns SWDGE
            # issue cost per dma_start dominates if fragmented), spread over
            # the sync (weights) and gpsimd (acts/biases) queues.
            b1_sb = wpool.tile([P, S * kt], mybir.dt.float32, tag="b1")
            nc.gpsimd.dma_start(out=b1_sb[:], in_=b1p[:])
            sb1_sb = wpool.tile([P, S], mybir.dt.float32, tag="sb1")
            nc.gpsimd.dma_start(out=sb1_sb[:], in_=sb1p[:])

            xr_sb = apool.tile([P, kt, GR], dt, tag="xr")
            nc.gpsimd.dma_start(out=xr_sb[:], in_=xr.rearrange("(k p) g -> p k g", p=P))

            w1_sb = {}
            w2_sb = {}
            for s in range(S):
                t = wpool.tile([P, kt, H], dt, tag=f"w1_{s}")
                nc.sync.dma_start(out=t[:], in_=w1[s].rearrange("(k p) h -> p k h", p=P))
                w1_sb[s] = t
                t = wpool.tile([P, kt, H], dt, tag=f"w2_{s}")
                nc.sync.dma_start(out=t[:], in_=w2[s].rearrange("(k p) h -> p k h", p=P))
                w2_sb[s] = t

            xs_sb = apool.tile([P, kt, GS], dt, tag="xs")
            nc.gpsimd.dma_start(out=xs_sb[:], in_=xs.rearrange("(k p) g -> p k g", p=P))
            sw1_sb = {}
            for s in range(S):
                t = wpool.tile([P, kt, F_SH], dt, tag=f"sw1_{s}")
                nc.gpsimd.dma_start(
                    out=t[:], in_=sw1[s].rearrange("(k p) f -> p k f", p=P)
                )
                sw1_sb[s] = t
            sw2_sb = wpool.tile([P, S, H], dt, tag="sw2")
            nc.gpsimd.dma_start(out=sw2_sb[:], in_=sw2.rearrange("s p h -> p s h"))

            # ---- routed path -----------------------------------------------
            for s in range(S):
                cs = slice(s * capR, (s + 1) * capR)
                h1_sb = []
                for f in range(kt):
                    ps = ppool.tile([P, capR], mybir.dt.float32, tag="ps1")
                    for k in range(kt):
                        nc.tensor.matmul(
                            ps,
                            w1_sb[s][:, k, f * P : (f + 1) * P],
                            xr_sb[:, k, cs],
                            start=(k == 0),
                            stop=(k == kt - 1),
                        )
                    h1 = apool.tile([P, capR], dt, tag=f"h1_{f}")
                    nc.scalar.activation(
                        h1[:], ps[:], GELU, bias=b1_sb[:, s * kt + f : s * kt + f + 1]
                    )
                    h1_sb.append(h1)
                o = opool.tile([P, kt, capR], OUT_DT, tag="yr_o")
                for hidx in range(kt):
                    ps = ppool.tile([P, capR], mybir.dt.float32, tag="ps2")
                    for f in range(kt):
                        nc.tensor.matmul(
                            ps,
                            w2_sb[s][:, f, hidx * P : (hidx + 1) * P],
                            h1_sb[f][:],
                            start=(f == 0),
                            stop=(f == kt - 1),
                        )
                    nc.vector.tensor_copy(o[:, hidx], ps[:])
                nc.gpsimd.dma_start(
                    out=yr.rearrange("(h p) g -> p h g", p=P)[:, :, cs], in_=o[:]
                )

            # ---- shared path -----------------------------------------------
            for s in range(S):
                cs = slice(s * capS, (s + 1) * capS)
                ps = ppool.tile([P, capS], mybir.dt.float32, tag="ps1")
                for k in range(kt):
                    nc.tensor.matmul(
                        ps,
                        sw1_sb[s][:, k],
                        xs_sb[:, k, cs],
                        start=(k == 0),
                        stop=(k == kt - 1),
                    )
                hs = apool.tile([P, capS], dt, tag="hs")
                nc.scalar.activation(hs[:], ps[:], GELU, bias=sb1_sb[:, s : s + 1])
                o = opool.tile([P, kt, capS], OUT_DT, tag="ys_o")
                for hidx in range(kt):
                    ps2 = ppool.tile([P, capS], mybir.dt.float32, tag="ps2")
                    nc.tensor.matmul(
                        ps2,
                        sw2_sb[:, s, hidx * P : (hidx + 1) * P],
                        hs[:],
                        start=True,
                        stop=True,
                    )
                    nc.vector.tensor_copy(o[:, hidx], ps2[:])
                nc.gpsimd.dma_start(
                    out=ys.rearrange("(h p) g -> p h g", p=P)[:, :, cs], in_=o[:]
                )

    nc.compile()
    return nc


def _sigmoid(v):
    out = np.empty_like(v)
    np.negative(np.abs(v), out=out)
    np.exp(out, out=out)
    pos = v >= 0
    out_pos = 1.0 / (1.0 + out)
    out_neg = out / (1.0 + out)
    return np.where(pos, out_pos, out_neg)


def kernel(**inputs) -> np.ndarray:
    x = np.ascontiguousarray(np.asarray(inputs["x"], dtype=np.float32))
    shared_W1 = np.asarray(inputs["shared_W1"], dtype=np.float32)
    shared_b1 = np.asarray(inputs["shared_b1"], dtype=np.float32)
    shared_W2 = np.asarray(inputs["shared_W2"], dtype=np.float32)
    shared_b2 = np.asarray(inputs["shared_b2"], dtype=np.float32)
    shared_router_W = np.asarray(inputs["shared_router_W"], dtype=np.float32)
    shared_router_b = np.asarray(inputs["shared_router_b"], dtype=np.float32)
    expert_W1 = np.asarray(inputs["expert_W1"], dtype=np.float32)
    expert_b1 = np.asarray(inputs["expert_b1"], dtype=np.float32)
    expert_W2 = np.asarray(inputs["expert_W2"], dtype=np.float32)
    expert_b2 = np.asarray(inputs["expert_b2"], dtype=np.float32)
    router_W = np.asarray(inputs["router_W"], dtype=np.float32)
    router_b = np.asarray(inputs["router_b"], dtype=np.float32)
    sub_router_W = np.asarray(inputs["sub_router_W"], dtype=np.float32)
    sub_router_b = np.asarray(inputs["sub_router_b"], dtype=np.float32)
    expert_bias = np.asarray(inputs["expert_bias"], dtype=np.float32)
    sub_expert_bias = np.asarray(inputs["sub_expert_bias"], dtype=np.float32)

    n = x.shape[0]
    assert x.shape == (N, H)

    # ---- host routing (matches reference's router math) --------------------
    sp = _sigmoid(x @ shared_router_W + shared_router_b + sub_expert_bias)  # [n,S]
    si = np.argmax(sp, axis=1)  # top-1 shared sub-expert
    sw = sp[np.arange(n), si]

    rp = _sigmoid(x @ router_W + router_b + expert_bias)  # [n,E]
    order2 = np.argsort(-rp, axis=1, kind="stable")[:, :2]  # top-2 experts
    ei = order2
    ew = np.take_along_axis(rp, ei, axis=1)  # [n,2]

    subp = _sigmoid(x @ sub_router_W + sub_router_b + sub_expert_bias)
    ssi = np.argmax(subp, axis=1)  # top-1 routed sub-expert (gate NOT applied)

    # ---- dispatch: group routed slots by (expert, sub-expert) --------------
    flat_tok = np.repeat(np.arange(n), 2)
    flat_e = ei.reshape(-1)
    flat_gate = ew.reshape(-1)
    flat_s = ssi[flat_tok]
    group = flat_e * S + flat_s
    counts = np.bincount(group, minlength=E * S)
    capR = max(64, _roundup(int(counts.max()), 16))

    sort_idx = np.argsort(group, kind="stable")
    g_tok = flat_tok[sort_idx]
    g_gate = flat_gate[sort_idx]
    g_off = np.concatenate([[0], np.cumsum(counts)])

    # shared groups: per core slice of 256 tokens, grouped by si
    capS_counts = []
    for c in range(NCORES):
        sl = si[c * N_PER_CORE : (c + 1) * N_PER_CORE]
        capS_counts.append(np.bincount(sl, minlength=S))
    capS_counts = np.stack(capS_counts)  # [NCORES, S]
    capS = max(32, _roundup(int(capS_counts.max()), 16))

    np_dt = _np_dt(MM_DT)
    xT = np.ascontiguousarray(x.T).astype(np_dt)  # [H, N]

    GR, GS = S * capR, S * capS
    in_maps = []
    tok_es = {}
    stok_cs = {}
    for c in range(NCORES):
        e = c
        xr_host = np.zeros((H, GR), dtype=np_dt)
        for s in range(S):
            g = e * S + s
            toks = g_tok[g_off[g] : g_off[g + 1]]
            tok_es[e, s] = (toks, g_gate[g_off[g] : g_off[g + 1]])
            xr_host[:, s * capR : s * capR + len(toks)] = xT[:, toks]

        xs_host = np.zeros((H, GS), dtype=np_dt)
        base = c * N_PER_CORE
        sl = si[base : base + N_PER_CORE]
        for s in range(S):
            toks = base + np.nonzero(sl == s)[0]
            stok_cs[c, s] = toks
            xs_host[:, s * capS : s * capS + len(toks)] = xT[:, toks]

        b1p = np.ascontiguousarray(
            expert_b1[e].reshape(S, H // P, P).transpose(2, 0, 1).reshape(P, -1)
        ).astype(np.float32)
        sb1p = np.ascontiguousarray(shared_b1.T).astype(np.float32)

        in_maps.append(
            {
                "xr": xr_host,
                "xs": xs_host,
                "w1": expert_W1[e].astype(np_dt),
                "w2": expert_W2[e].astype(np_dt),
                "sw1": shared_W1.astype(np_dt),
                "sw2": shared_W2.astype(np_dt),
                "b1p": b1p,
                "sb1p": sb1p,
            }
        )

    key = (capR, capS, MM_DT)
    nc = _compiled_cache.get(key)
    if nc is None:
        nc = _build_module(capR, capS)
        _compiled_cache[key] = nc

    res = run_bass_kernel_spmd(nc, in_maps, core_ids=list(range(NCORES)))
    global _last_res
    _last_res = res

    # ---- host combine ------------------------------------------------------
    out = np.zeros((N, H), dtype=np.float32)
    for c in range(NCORES):
        e = c
        yr_out = res.results[c]["yr"]  # [H, GR]
        ys_out = res.results[c]["ys"]  # [H, GS]
        for s in range(S):
            toks, gates = tok_es[e, s]
            if len(toks):
                ycols = yr_out[:, s * capR : s * capR + len(toks)].T  # [cnt, H]
                out[toks] += gates[:, None] * (ycols + expert_b2[e, s])
            stoks = stok_cs[c, s]
            if len(stoks):
                ycols = ys_out[:, s * capS : s * capS + len(stoks)].T
                out[stoks] += sw[stoks, None] * (ycols + shared_b2[s])

    return out


# revision 13
# speedup vs baseline: 1.8992x; 1.8992x over previous
"""DeepSeekMoE Trainium2 kernel: expert-parallel across 8 NeuronCores.

Strategy:
  - Host computes routing (3 small sigmoid routers + top-k) and performs the
    token all-to-all: for each expert e (= core e), gather the tokens that
    chose e in their top-2, grouped by their top-1 sub-expert, padded to a
    static capacity.  Tokens ship transposed ([H, tokens]) so the two matmul
    chains need no on-chip transposes:
        H1^T = W1^T @ X^T   (gelu + b1 fused on ScalarE)
        Y^T  = W2^T @ H1^T
    Weights are the stationary PE operand in their natural [in, out] layout.
  - The shared-expert path is data-parallel: core c processes tokens
    [c*256, (c+1)*256), grouped by top-1 shared sub-expert.
  - Host applies the second bias + sigmoid gates and scatter-adds back.
Only the routing/dispatch (<1% of FLOPs) runs on host; both FFN chains for
every selected (expert, sub-expert) combo run on device.
"""

import sys

sys.path.insert(0, "/opt/trn_rl_repo")

import numpy as np

import concourse.bass as bass  # noqa: F401  (registers AP machinery)
import concourse.mybir as mybir
from concourse import bacc
from concourse.tile import TileContext
from concourse.bass_utils import run_bass_kernel_spmd

N, H, E, S = 2048, 512, 8, 4
F_SH = 128
P = 128
NCORES = 8
N_PER_CORE = N // NCORES

# Device compute dtype for matmul operands.  fp16 runs the PE at 1 cycle/row
# (4x faster than fp32) with fp32 PSUM accumulation; with |x|<~6 and |w|<~0.2
# there is no overflow risk and ~2^-11 mantissa keeps rel-err ~1e-4.
MM_DT = mybir.dt.float16
OUT_DT = mybir.dt.float16  # device->host y dtype (fp32 PSUM quantized once)

_compiled_cache: dict = {}
_last_res = None


def _roundup(v: int, m: int) -> int:
    return ((v + m - 1) // m) * m


def _np_dt(dt):
    return mybir.dt.np(dt)


def _build_module(capR: int, capS: int):
    """Build + compile the SPMD per-core Bass module."""
    GR = S * capR
    GS = S * capS
    kt = H // P  # 4 k-tiles over the hidden dim

    nc = bacc.Bacc("TRN2", target_bir_lowering=False, debug=False)
    dt = MM_DT

    xr = nc.dram_tensor("xr", [H, GR], dt, kind="ExternalInput").ap()
    xs = nc.dram_tensor("xs", [H, GS], dt, kind="ExternalInput").ap()
    w1 = nc.dram_tensor("w1", [S, H, H], dt, kind="ExternalInput").ap()
    w2 = nc.dram_tensor("w2", [S, H, H], dt, kind="ExternalInput").ap()
    sw1 = nc.dram_tensor("sw1", [S, H, F_SH], dt, kind="ExternalInput").ap()
    sw2 = nc.dram_tensor("sw2", [S, F_SH, H], dt, kind="ExternalInput").ap()
    # b1 packed on host: b1p[p, s*kt + f] = b1[s, f*128 + p]
    b1p = nc.dram_tensor("b1p", [P, S * kt], mybir.dt.float32, kind="ExternalInput").ap()
    # sb1 packed: sb1p[p, s] = sb1[s, p]
    sb1p = nc.dram_tensor("sb1p", [P, S], mybir.dt.float32, kind="ExternalInput").ap()

    yr = nc.dram_tensor("yr", [H, GR], OUT_DT, kind="ExternalOutput").ap()
    ys = nc.dram_tensor("ys", [H, GS], OUT_DT, kind="ExternalOutput").ap()

    GELU = mybir.ActivationFunctionType.Gelu

    with TileContext(nc) as tc:
        with (
            tc.tile_pool(name="weights", bufs=1) as wpool,
            tc.tile_pool(name="acts", bufs=1) as apool,
            tc.tile_pool(name="outs", bufs=2) as opool,
            tc.tile_pool(name="psum", bufs=4, space="PSUM") as ppool,
        ):
            # ---- loads: one dma_start per tensor/group (the ~600ns SWDGE
            # issue cost per dma_start dominates if fragmented), spread over
            # the sync (weights) and gpsimd (acts/biases) queues.
            b1_sb = wpool.tile([P, S * kt], mybir.dt.float32, tag="b1")
            nc.gpsimd.dma_start(out=b1_sb[:], in_=b1p[:])
            sb1_sb = wpool.tile([P, S], mybir.dt.float32, tag="sb1")
            nc.gpsimd.dma_start(out=sb1_sb[:], in_=sb1p[:])

            xr_sb = apool.tile([P, kt, GR], dt, tag="xr")
            nc.gpsimd.dma_start(out=xr_sb[:], in_=xr.rearrange("(k p) g -> p k g", p=P))

            w1_sb = {}
            w2_sb = {}
            for s in range(S):
                t = wpool.tile([P, kt, H], dt, tag=f"w1_{s}")
                nc.sync.dma_start(out=t[:], in_=w1[s].rearrange("(k p) h -> p k h", p=P))
                w1_sb[s] = t
                t = wpool.tile([P, kt, H], dt, tag=f"w2_{s}")
                nc.sync.dma_start(out=t[:], in_=w2[s].rearrange("(k p) h -> p k h", p=P))
                w2_sb[s] = t

            xs_sb = apool.tile([P, kt, GS], dt, tag="xs")
            nc.gpsimd.dma_start(out=xs_sb[:], in_=xs.rearrange("(k p) g -> p k g", p=P))
            sw1_sb = {}
            for s in range(S):
                t = wpool.tile([P, kt, F_SH], dt, tag=f"sw1_{s}")
                nc.gpsimd.dma_start(
                    out=t[:], in_=sw1[s].rearrange("(k p) f -> p k f", p=P)
                )
                sw1_sb[s] = t
            sw2_sb = wpool.tile([P, S, H], dt, tag="sw2")
            nc.gpsimd.dma_start(out=sw2_sb[:], in_=sw2.rearrange("s p h -> p s h"))

            # ---- routed path -----------------------------------------------
            for s in range(S):
                cs = slice(s * capR, (s + 1) * capR)
                h1_sb = []
                for f in range(kt):
                    ps = ppool.tile([P, capR], mybir.dt.float32, tag="ps1")
                    for k in range(kt):
                        nc.tensor.matmul(
                            ps,
                            w1_sb[s][:, k, f * P : (f + 1) * P],
                            xr_sb[:, k, cs],
                            start=(k == 0),
                            stop=(k == kt - 1),
                        )
                    h1 = apool.tile([P, capR], dt, tag=f"h1_{f}")
                    nc.scalar.activation(
                        h1[:], ps[:], GELU, bias=b1_sb[:, s * kt + f : s * kt + f + 1]
                    )
                    h1_sb.append(h1)
                o = opool.tile([P, kt, capR], OUT_DT, tag="yr_o")
                for hidx in range(kt):
                    ps = ppool.tile([P, capR], mybir.dt.float32, tag="ps2")
                    for f in range(kt):
                        nc.tensor.matmul(
                            ps,
                            w2_sb[s][:, f, hidx * P : (hidx + 1) * P],
                            h1_sb[f][:],
                            start=(f == 0),
                            stop=(f == kt - 1),
                        )
                    nc.vector.tensor_copy(o[:, hidx], ps[:])
                nc.gpsimd.dma_start(
                    out=yr.rearrange("(h p) g -> p h g", p=P)[:, :, cs], in_=o[:]
                )

            # ---- shared path -----------------------------------------------
            for s in range(S):
                cs = slice(s * capS, (s + 1) * capS)
                ps = ppool.tile([P, capS], mybir.dt.float32, tag="ps1")
                for k in range(kt):
                    nc.tensor.matmul(
                        ps,
                        sw1_sb[s][:, k],
                        xs_sb[:, k, cs],
                        start=(k == 0),
                        stop=(k == kt - 1),
                    )
                hs = apool.tile([P, capS], dt, tag="hs")
                nc.scalar.activation(hs[:], ps[:], GELU, bias=sb1_sb[:, s : s + 1])
                o = opool.tile([P, kt, capS], OUT_DT, tag="ys_o")
                for hidx in range(kt):
                    ps2 = ppool.tile([P, capS], mybir.dt.float32, tag="ps2")
                    nc.tensor.matmul(
                        ps2,
                        sw2_sb[:, s, hidx * P : (hidx + 1) * P],
                        hs[:],
                        start=True,
                        stop=True,
                    )
                    nc.vector.tensor_copy(o[:, hidx], ps2[:])
                nc.gpsimd.dma_start(
                    out=ys.rearrange("(h p) g -> p h g", p=P)[:, :, cs], in_=o[:]
                )

    nc.compile()
    return nc


def _sigmoid(v):
    out = np.empty_like(v)
    np.negative(np.abs(v), out=out)
    np.exp(out, out=out)
    pos = v >= 0
    out_pos = 1.0 / (1.0 + out)
    out_neg = out / (1.0 + out)
    return np.where(pos, out_pos, out_neg)


def kernel(**inputs) -> np.ndarray:
    x = np.ascontiguousarray(np.asarray(inputs["x"], dtype=np.float32))
    shared_W1 = np.asarray(inputs["shared_W1"], dtype=np.float32)
    shared_b1 = np.asarray(inputs["shared_b1"], dtype=np.float32)
    shared_W2 = np.asarray(inputs["shared_W2"], dtype=np.float32)
    shared_b2 = np.asarray(inputs["shared_b2"], dtype=np.float32)
    shared_router_W = np.asarray(inputs["shared_router_W"], dtype=np.float32)
    shared_router_b = np.asarray(inputs["shared_router_b"], dtype=np.float32)
    expert_W1 = np.asarray(inputs["expert_W1"], dtype=np.float32)
    expert_b1 = np.asarray(inputs["expert_b1"], dtype=np.float32)
    expert_W2 = np.asarray(inputs["expert_W2"], dtype=np.float32)
    expert_b2 = np.asarray(inputs["expert_b2"], dtype=np.float32)
    router_W = np.asarray(inputs["router_W"], dtype=np.float32)
    router_b = np.asarray(inputs["router_b"], dtype=np.float32)
    sub_router_W = np.asarray(inputs["sub_router_W"], dtype=np.float32)
    sub_router_b = np.asarray(inputs["sub_router_b"], dtype=np.float32)
    expert_bias = np.asarray(inputs["expert_bias"], dtype=np.float32)
    sub_expert_bias = np.asarray(inputs["sub_expert_bias"], dtype=np.float32)

    n = x.shape[0]
    assert x.shape == (N, H)

    # ---- host routing (matches reference's router math) --------------------
    sp = _sigmoid(x @ shared_router_W + shared_router_b + sub_expert_bias)  # [n,S]
    si = np.argmax(sp, axis=1)  # top-1 shared sub-expert
    sw = sp[np.arange(n), si]

    rp = _sigmoid(x @ router_W + router_b + expert_bias)  # [n,E]
    order2 = np.argsort(-rp, axis=1, kind="stable")[:, :2]  # top-2 experts
    ei = order2
    ew = np.take_along_axis(rp, ei, axis=1)  # [n,2]

    subp = _sigmoid(x @ sub_router_W + sub_router_b + sub_expert_bias)
    ssi = np.argmax(subp, axis=1)  # top-1 routed sub-expert (gate NOT applied)

    # ---- dispatch: group routed slots by (expert, sub-expert) --------------
    flat_tok = np.repeat(np.arange(n), 2)
    flat_e = ei.reshape(-1)
    flat_gate = ew.reshape(-1)
    flat_s = ssi[flat_tok]
    group = flat_e * S + flat_s
    counts = np.bincount(group, minlength=E * S)
    capR = max(64, _roundup(int(counts.max()), 16))

    sort_idx = np.argsort(group, kind="stable")
    g_tok = flat_tok[sort_idx]
    g_gate = flat_gate[sort_idx]
    g_off = np.concatenate([[0], np.cumsum(counts)])

    # shared groups: per core slice of 256 tokens, grouped by si
    capS_counts = []
    for c in range(NCORES):
        sl = si[c * N_PER_CORE : (c + 1) * N_PER_CORE]
        capS_counts.append(np.bincount(sl, minlength=S))
    capS_counts = np.stack(capS_counts)  # [NCORES, S]
    capS = max(32, _roundup(int(capS_counts.max()), 16))

    np_dt = _np_dt(MM_DT)
    xT = np.ascontiguousarray(x.T).astype(np_dt)  # [H, N]

    GR, GS = S * capR, S * capS
    in_maps = []
    tok_es = {}
    stok_cs = {}
    for c in range(NCORES):
        e = c
        xr_host = np.zeros((H, GR), dtype=np_dt)
        for s in range(S):
            g = e * S + s
            toks = g_tok[g_off[g] : g_off[g + 1]]
            tok_es[e, s] = (toks, g_gate[g_off[g] : g_off[g + 1]])
            xr_host[:, s * capR : s * capR + len(toks)] = xT[:, toks]

        xs_host = np.zeros((H, GS), dtype=np_dt)
        base = c * N_PER_CORE
        sl = si[base : base + N_PER_CORE]
        for s in range(S):
            toks = base + np.nonzero(sl == s)[0]
            stok_cs[c, s] = toks
            xs_host[:, s * capS : s * capS + len(toks)] = xT[:, toks]

        b1p = np.ascontiguousarray(
            expert_b1[e].reshape(S, H // P, P).transpose(2, 0, 1).reshape(P, -1)
        ).astype(np.float32)
        sb1p = np.ascontiguousarray(shared_b1.T).astype(np.float32)

        in_maps.append(
            {
                "xr": xr_host,
                "xs": xs_host,
                "w1": expert_W1[e].astype(np_dt),
                "w2": expert_W2[e].astype(np_dt),
                "sw1": shared_W1.astype(np_dt),
                "sw2": shared_W2.astype(np_dt),
                "b1p": b1p,
                "sb1p": sb1p,
            }
        )

    key = (capR, capS, MM_DT)
    nc = _compiled_cache.get(key)
    if nc is None:
        import time as _time

        _t = _time.time()
        nc = _build_module(capR, capS)
        print(f"[kernel] built module capR={capR} capS={capS} "
              f"in {_time.time() - _t:.1f}s", flush=True)
        _compiled_cache[key] = nc

    res = run_bass_kernel_spmd(nc, in_maps, core_ids=list(range(NCORES)))
    global _last_res
    _last_res = res

    # ---- host combine ------------------------------------------------------
    out = np.zeros((N, H), dtype=np.float32)
    for c in range(NCORES):
        e = c
        yr_out = res.results[c]["yr"]  # [H, GR]
        ys_out = res.results[c]["ys"]  # [H, GS]
        for s in range(S):
            toks, gates = tok_es[e, s]
            if len(toks):
                ycols = yr_out[:, s * capR : s * capR + len(toks)].T  # [cnt, H]
                out[toks] += gates[:, None] * (ycols + expert_b2[e, s])
            stoks = stok_cs[c, s]
            if len(stoks):
                ycols = ys_out[:, s * capS : s * capS + len(stoks)].T
                out[stoks] += sw[stoks, None] * (ycols + shared_b2[s])

    return out
